# revision 6
# baseline (speedup 1.0000x reference)
"""DualRelGCN message-passing kernel for 8 TRN2 NeuronCores.

Strategy (destination-sharded, collective-free):
  - The output row for node d depends on (a) rel_embed[d] and (b) the weighted
    mean over edges with dst==d.  LayerNorm is invariant to positive per-row
    scaling, so LN(agg/denom) == LN(agg): the denominator drops out entirely.
  - Shard edges by dst range: core c owns nodes [1250c, 1250(c+1)) and receives
    every edge whose dst falls there (balanced for random graphs).  Each core
    computes its 1250 output rows completely locally -> no collectives.
  - Per core: edges are sorted by dst and packed into 128-edge chunks grouped
    by 128-node destination tiles.  A hardware dma_gather pulls the bf16 src
    rows (msg), a per-chunk one-hot(weight) matrix is built on DVE with a
    single dual-op tensor_scalar (is_equal x weight), and PE matmuls
    onehotw.T @ msg accumulate the segment sums for each dst tile in PSUM.
  - Epilogue per dst tile: LN on DVE/ACT, PE transpose, y = ln @ proj_w.T on
    PE, out = rel_embed + 0.1*y.
"""

import sys

for _p in ("/opt/trn_rl_repo",):
    if _p not in sys.path:
        sys.path.insert(0, _p)

import numpy as np
import ml_dtypes

import concourse.bacc as bacc
import concourse.bass as bass
import concourse.mybir as mybir
from concourse.alu_op_type import AluOpType
from concourse.tile import TileContext
from concourse.bass_utils import run_bass_kernel_spmd

F32 = mybir.dt.float32
BF16 = mybir.dt.bfloat16
I16 = mybir.dt.int16
AF = mybir.ActivationFunctionType

N_NODES = 10000
DIM = 256
N_CORES = 8
NODES_PER_CORE = N_NODES // N_CORES  # 1250
TILE = 128
N_TILES = -(-NODES_PER_CORE // TILE)  # 10
OUT_ROWS = N_TILES * TILE  # 1280
ALPHA = 0.1
LN_EPS = 1e-5

_CACHE: dict = {}


def _build(kmax: int):
    """Build the SPMD program. Structure depends only on kmax (chunks per
    dst tile); all data-dependent content arrives via input tensors."""
    C = N_TILES * kmax  # total 128-edge chunks per core

    nc = bacc.Bacc("TRN2", target_bir_lowering=False, debug=False,
                   num_devices=N_CORES)

    rel16 = nc.dram_tensor("rel16", [N_NODES, DIM], BF16, kind="ExternalInput")
    idx_d = nc.dram_tensor("idx", [128, C * 8], I16, kind="ExternalInput")
    dstrel_d = nc.dram_tensor("dstrel", [128, C], F32, kind="ExternalInput")
    w_d = nc.dram_tensor("w", [128, C], F32, kind="ExternalInput")
    relsl_d = nc.dram_tensor("relslice", [OUT_ROWS, DIM], F32,
                             kind="ExternalInput")
    pwt_d = nc.dram_tensor("projwT", [128, 2 * DIM], BF16, kind="ExternalInput")
    out_d = nc.dram_tensor("out", [OUT_ROWS, DIM], F32, kind="ExternalOutput")

    from contextlib import ExitStack
    with TileContext(nc) as tc, ExitStack() as es:
        const_pool = es.enter_context(tc.tile_pool(name="const", bufs=1))
        msg_pool = es.enter_context(tc.tile_pool(name="msg", bufs=3))
        oh_pool = es.enter_context(tc.tile_pool(name="oh", bufs=6))
        ep_pool = es.enter_context(tc.tile_pool(name="ep", bufs=2))
        ps_agg = es.enter_context(tc.tile_pool(name="ps_agg", bufs=2,
                                               space="PSUM"))
        ps_tr = es.enter_context(tc.tile_pool(name="ps_tr", bufs=2,
                                              space="PSUM"))
        ps_y = es.enter_context(tc.tile_pool(name="ps_y", bufs=2,
                                             space="PSUM"))

        # --- constants / inputs resident in SBUF ---
        iota_row = const_pool.tile([128, 128], F32, tag="iota")
        nc.gpsimd.iota(iota_row[:], [[1, 128]], base=0, channel_multiplier=0,
                       allow_small_or_imprecise_dtypes=True)
        pidx = const_pool.tile([128, 1], F32, tag="pidx")
        nc.gpsimd.iota(pidx[:], [[1, 1]], base=0, channel_multiplier=1,
                       allow_small_or_imprecise_dtypes=True)
        ident = const_pool.tile([128, 128], BF16, tag="ident")
        nc.vector.tensor_scalar(ident[:], iota_row[:], pidx[:], None,
                                AluOpType.is_equal)
        epsb = const_pool.tile([128, 1], F32, tag="epsb")
        nc.vector.memset(epsb[:], LN_EPS)

        idx_sb = const_pool.tile([128, C * 8], I16, tag="idx")
        nc.sync.dma_start(idx_sb[:], idx_d[:])
        dstrel_sb = const_pool.tile([128, C], F32, tag="dstrel")
        nc.sync.dma_start(dstrel_sb[:], dstrel_d[:])
        w_sb = const_pool.tile([128, C], F32, tag="w")
        nc.sync.dma_start(w_sb[:], w_d[:])
        pwt_sb = const_pool.tile([128, 2, DIM], BF16, tag="pwt")
        nc.sync.dma_start(pwt_sb[:], pwt_d[:])

        for t in range(N_TILES):
            # --- gather the bf16 src rows for this dst tile's edges ---
            msg = msg_pool.tile([128, kmax, DIM], BF16, tag="msg")
            nc.gpsimd.dma_gather(
                msg[:], rel16[:],
                idx_sb[:, t * kmax * 8:(t + 1) * kmax * 8],
                num_idxs=kmax * 128, num_idxs_reg=kmax * 128, elem_size=DIM,
                single_packet=False,
            )

            # --- segment-sum into PSUM via one-hot(weight) matmuls ---
            agg_ps = ps_agg.tile([128, DIM], F32, tag="agg")
            for g in range(kmax):
                c = t * kmax + g
                oh = oh_pool.tile([128, 128], BF16, tag="oh")
                nc.vector.tensor_scalar(
                    oh[:], iota_row[:], dstrel_sb[:, c:c + 1],
                    w_sb[:, c:c + 1], AluOpType.is_equal, AluOpType.mult)
                nc.tensor.matmul(agg_ps[:], oh[:], msg[:, g, :],
                                 start=(g == 0), stop=(g == kmax - 1))

            # --- epilogue: LN -> transpose -> @ proj_w.T -> residual ---
            agg = ep_pool.tile([128, DIM], F32, tag="agg_sb")
            rowsum = ep_pool.tile([128, 1], F32, tag="rowsum")
            nc.scalar.activation(agg[:], agg_ps[:], AF.Copy,
                                 accum_out=rowsum[:])
            mean = ep_pool.tile([128, 1], F32, tag="mean")
            nc.scalar.mul(mean[:], rowsum[:], 1.0 / DIM)
            cent = ep_pool.tile([128, DIM], F32, tag="cent")
            nc.vector.tensor_scalar(cent[:], agg[:], mean[:], None,
                                    AluOpType.subtract)
            sq = ep_pool.tile([128, DIM], F32, tag="sq")
            sumsq = ep_pool.tile([128, 1], F32, tag="sumsq")
            nc.scalar.activation(sq[:], cent[:], AF.Square,
                                 accum_out=sumsq[:])
            std = ep_pool.tile([128, 1], F32, tag="std")
            nc.scalar.activation(std[:], sumsq[:], AF.Sqrt, bias=epsb[:],
                                 scale=1.0 / DIM)
            rstd = ep_pool.tile([128, 1], F32, tag="rstd")
            nc.vector.reciprocal(rstd[:], std[:])
            ln = ep_pool.tile([128, DIM], BF16, tag="ln")
            nc.vector.tensor_scalar(ln[:], cent[:], rstd[:], None,
                                    AluOpType.mult)

            y_ps = ps_y.tile([128, DIM], F32, tag="y")
            for k in range(2):
                tr_ps = ps_tr.tile([128, 128], BF16, tag="tr")
                nc.tensor.transpose(tr_ps[:], ln[:, k * 128:(k + 1) * 128],
                                    ident[:])
                lnT = ep_pool.tile([128, 128], BF16, tag="lnT")
                nc.scalar.copy(lnT[:], tr_ps[:])
                nc.tensor.matmul(y_ps[:], lnT[:], pwt_sb[:, k, :],
                                 start=(k == 0), stop=(k == 1))

            rel_t = ep_pool.tile([128, DIM], F32, tag="rel")
            nc.sync.dma_start(rel_t[:], relsl_d[t * 128:(t + 1) * 128, :])
            delta = ep_pool.tile([128, DIM], F32, tag="delta")
            nc.vector.tensor_scalar(delta[:], y_ps[:], ALPHA, None,
                                    AluOpType.mult)
            out_t = ep_pool.tile([128, DIM], F32, tag="out")
            nc.vector.tensor_tensor(out_t[:], delta[:], rel_t[:],
                                    AluOpType.add)
            nc.sync.dma_start(out_d[t * 128:(t + 1) * 128, :], out_t[:])

    nc.compile()
    return nc


def _prep(rel_embed, rel_edge_index, rel_edge_weight, proj_w):
    """Host-side sharding/layout: sort edges by dst, shard by dst range,
    pack into uniform chunks-per-tile, build device input maps."""
    src = np.asarray(rel_edge_index[0], dtype=np.int64)
    dst = np.asarray(rel_edge_index[1], dtype=np.int64)
    w = np.asarray(rel_edge_weight, dtype=np.float32)
    rel = np.asarray(rel_embed, dtype=np.float32)
    pw = np.asarray(proj_w, dtype=np.float32)

    order = np.argsort(dst, kind="stable")
    dst_s, src_s, w_s = dst[order], src[order], w[order]

    # boundaries of every (core, tile) segment in the dst-sorted edge list
    tile_lo = []
    for c in range(N_CORES):
        base = c * NODES_PER_CORE
        for t in range(N_TILES):
            tile_lo.append(base + min(t * TILE, NODES_PER_CORE))
    tile_lo.append(N_NODES)
    bounds = np.searchsorted(dst_s, np.array(tile_lo))
    cnt = np.diff(bounds).reshape(N_CORES, N_TILES)

    kmax = max(1, int(np.max(-(-cnt // 128))))
    C = N_TILES * kmax

    idx16 = np.zeros((N_CORES, C * 128), dtype=np.int16)
    dstrel = np.zeros((N_CORES, C * 128), dtype=np.float32)
    warr = np.zeros((N_CORES, C * 128), dtype=np.float32)
    for c in range(N_CORES):
        for t in range(N_TILES):
            b0 = bounds[c * N_TILES + t]
            n = cnt[c, t]
            o = t * kmax * 128
            idx16[c, o:o + n] = src_s[b0:b0 + n]
            dstrel[c, o:o + n] = (dst_s[b0:b0 + n]
                                  - (c * NODES_PER_CORE + t * TILE))
            warr[c, o:o + n] = w_s[b0:b0 + n]

    # device layouts
    # idx: per-gather-stage (= per dst tile) 16-partition wrap, replicated x8
    idx_dev = np.zeros((N_CORES, 128, C * 8), dtype=np.int16)
    for t in range(N_TILES):
        blk = idx16[:, t * kmax * 128:(t + 1) * kmax * 128]
        wrap = blk.reshape(N_CORES, kmax * 8, 16).transpose(0, 2, 1)
        idx_dev[:, :, t * kmax * 8:(t + 1) * kmax * 8] = np.tile(
            wrap, (1, 8, 1))
    # per-edge scalars: edge j of chunk c sits at [j % 128, c]
    dstrel_dev = np.ascontiguousarray(
        dstrel.reshape(N_CORES, C, 128).transpose(0, 2, 1))
    w_dev = np.ascontiguousarray(
        warr.reshape(N_CORES, C, 128).transpose(0, 2, 1))

    rel16 = rel.astype(ml_dtypes.bfloat16)
    relslice = np.zeros((N_CORES, OUT_ROWS, DIM), dtype=np.float32)
    for c in range(N_CORES):
        relslice[c, :NODES_PER_CORE] = rel[c * NODES_PER_CORE:
                                           (c + 1) * NODES_PER_CORE]
    pwt = pw.T.astype(ml_dtypes.bfloat16)  # [f, o]
    pwt_dev = np.ascontiguousarray(
        pwt.reshape(2, 128, DIM).transpose(1, 0, 2).reshape(128, 2 * DIM))

    in_maps = []
    for c in range(N_CORES):
        in_maps.append({
            "rel16": rel16,
            "idx": idx_dev[c],
            "dstrel": dstrel_dev[c],
            "w": w_dev[c],
            "relslice": relslice[c],
            "projwT": pwt_dev,
        })
    return kmax, in_maps


def kernel(rel_embed, rel_edge_index, rel_edge_weight, proj_w,
           _trace=False):
    kmax, in_maps = _prep(rel_embed, rel_edge_index, rel_edge_weight, proj_w)
    nc = _CACHE.get(kmax)
    if nc is None:
        nc = _build(kmax)
        _CACHE[kmax] = nc
    res = run_bass_kernel_spmd(nc, in_maps, core_ids=list(range(N_CORES)),
                               trace=_trace)
    out = np.concatenate(
        [res.results[c]["out"][:NODES_PER_CORE] for c in range(N_CORES)],
        axis=0)
    if _trace:
        kernel.last_results = res
    return out.astype(np.float32)


# revision 7
# speedup vs baseline: 3.0612x; 3.0612x over previous
"""DualRelGCN message-passing kernel for 8 TRN2 NeuronCores.

Strategy (destination-sharded, collective-free, block-dense):
  - LayerNorm is invariant to positive per-row scaling, so LN(agg/denom) ==
    LN(agg): the denominator drops out of the computation entirely.
  - Shard edges by dst range: core c owns nodes [1250c, 1250(c+1)) and
    receives every edge whose dst falls there.  Each core computes its 1250
    output rows completely locally -> no collectives.
  - The weighted gather+segment_sum is expressed as a block matmul:
    agg[tile t] = sum_s W_ts.T @ X_s, where W_ts is the [128 src, 128 dst]
    dense block of the weighted adjacency (host-scattered from the edge
    list; ~5% nnz but dense matmul on PE beats any descriptor-generated
    gather path by a wide margin) and X_s is a [128, 256] tile of rel_embed
    (bf16, fully resident in SBUF).  PSUM accumulates over s in fp32.
  - Epilogue per dst tile: LN on DVE/ACT, PE transpose, y = ln @ proj_w.T,
    out = rel_embed + 0.1*y.
  - The device program is fully static: the edge distribution only changes
    tensor *contents*, never the instruction stream.
"""

import sys

for _p in ("/opt/trn_rl_repo",):
    if _p not in sys.path:
        sys.path.insert(0, _p)

from contextlib import ExitStack

import numpy as np
import ml_dtypes

import concourse.bacc as bacc
import concourse.mybir as mybir
from concourse.alu_op_type import AluOpType
from concourse.tile import TileContext
from concourse.bass_utils import run_bass_kernel_spmd

F32 = mybir.dt.float32
BF16 = mybir.dt.bfloat16
AF = mybir.ActivationFunctionType

N_NODES = 10000
DIM = 256
N_CORES = 8
NODES_PER_CORE = N_NODES // N_CORES  # 1250
TILE = 128
N_TILES = -(-NODES_PER_CORE // TILE)  # 10 dst tiles per core
S_TILES = -(-N_NODES // TILE)  # 79 src tiles
OUT_ROWS = N_TILES * TILE  # 1280
ALPHA = 0.1
LN_EPS = 1e-5
X_CHUNKS = 8  # X load split for pipelined startup

_CACHE: dict = {}


def _build():
    nc = bacc.Bacc("TRN2", target_bir_lowering=False, debug=False,
                   num_devices=N_CORES)

    x_d = nc.dram_tensor("x", [128, S_TILES * DIM], BF16, kind="ExternalInput")
    w_d = nc.dram_tensor("wblk", [N_TILES, 128, S_TILES * TILE], BF16,
                         kind="ExternalInput")
    relsl_d = nc.dram_tensor("relslice", [OUT_ROWS, DIM], F32,
                             kind="ExternalInput")
    pwt_d = nc.dram_tensor("projwT", [128, 2 * DIM], BF16,
                           kind="ExternalInput")
    out_d = nc.dram_tensor("out", [OUT_ROWS, DIM], F32, kind="ExternalOutput")

    with TileContext(nc) as tc, ExitStack() as es:
        const_pool = es.enter_context(tc.tile_pool(name="const", bufs=1))
        wpool = es.enter_context(tc.tile_pool(name="wblk", bufs=2))
        ep_pool = es.enter_context(tc.tile_pool(name="ep", bufs=2))
        ps_agg = es.enter_context(tc.tile_pool(name="ps_agg", bufs=2,
                                               space="PSUM"))
        ps_tr = es.enter_context(tc.tile_pool(name="ps_tr", bufs=2,
                                              space="PSUM"))
        ps_y = es.enter_context(tc.tile_pool(name="ps_y", bufs=2,
                                             space="PSUM"))

        # --- constants / resident inputs ---
        iota_row = const_pool.tile([128, 128], F32, tag="iota")
        nc.gpsimd.iota(iota_row[:], [[1, 128]], base=0, channel_multiplier=0,
                       allow_small_or_imprecise_dtypes=True)
        pidx = const_pool.tile([128, 1], F32, tag="pidx")
        nc.gpsimd.iota(pidx[:], [[1, 1]], base=0, channel_multiplier=1,
                       allow_small_or_imprecise_dtypes=True)
        ident = const_pool.tile([128, 128], BF16, tag="ident")
        nc.vector.tensor_scalar(ident[:], iota_row[:], pidx[:], None,
                                AluOpType.is_equal)
        epsb = const_pool.tile([128, 1], F32, tag="epsb")
        nc.vector.memset(epsb[:], LN_EPS)
        pwt_sb = const_pool.tile([128, 2, DIM], BF16, tag="pwt")
        nc.sync.dma_start(pwt_sb[:], pwt_d[:])

        # rel_embed (bf16), fully resident; chunked load so dst-tile 0's
        # matmuls can start before the whole 5 MB lands
        x_sb = const_pool.tile([128, S_TILES, DIM], BF16, tag="x")
        bounds = [round(i * S_TILES / X_CHUNKS) for i in range(X_CHUNKS + 1)]
        for i in range(X_CHUNKS):
            lo, hi = bounds[i], bounds[i + 1]
            nc.sync.dma_start(x_sb[:, lo:hi, :],
                              x_d[:, lo * DIM:hi * DIM])

        for t in range(N_TILES):
            w_t = wpool.tile([128, S_TILES, TILE], BF16, tag="w")
            nc.sync.dma_start(w_t[:], w_d[t])

            agg_ps = ps_agg.tile([128, DIM], F32, tag="agg")
            for s in range(S_TILES):
                nc.tensor.matmul(agg_ps[:], w_t[:, s, :], x_sb[:, s, :],
                                 start=(s == 0), stop=(s == S_TILES - 1))

            # --- epilogue: LN -> transpose -> @ proj_w.T -> residual ---
            agg = ep_pool.tile([128, DIM], F32, tag="agg_sb")
            rowsum = ep_pool.tile([128, 1], F32, tag="rowsum")
            nc.scalar.activation(agg[:], agg_ps[:], AF.Copy,
                                 accum_out=rowsum[:])
            mean = ep_pool.tile([128, 1], F32, tag="mean")
            nc.scalar.mul(mean[:], rowsum[:], 1.0 / DIM)
            cent = ep_pool.tile([128, DIM], F32, tag="cent")
            nc.vector.tensor_scalar(cent[:], agg[:], mean[:], None,
                                    AluOpType.subtract)
            sq = ep_pool.tile([128, DIM], F32, tag="sq")
            sumsq = ep_pool.tile([128, 1], F32, tag="sumsq")
            nc.scalar.activation(sq[:], cent[:], AF.Square,
                                 accum_out=sumsq[:])
            std = ep_pool.tile([128, 1], F32, tag="std")
            nc.scalar.activation(std[:], sumsq[:], AF.Sqrt, bias=epsb[:],
                                 scale=1.0 / DIM)
            rstd = ep_pool.tile([128, 1], F32, tag="rstd")
            nc.vector.reciprocal(rstd[:], std[:])
            ln = ep_pool.tile([128, DIM], BF16, tag="ln")
            nc.vector.tensor_scalar(ln[:], cent[:], rstd[:], None,
                                    AluOpType.mult)

            y_ps = ps_y.tile([128, DIM], F32, tag="y")
            for k in range(2):
                tr_ps = ps_tr.tile([128, 128], BF16, tag="tr")
                nc.tensor.transpose(tr_ps[:], ln[:, k * 128:(k + 1) * 128],
                                    ident[:])
                lnT = ep_pool.tile([128, 128], BF16, tag="lnT")
                nc.scalar.copy(lnT[:], tr_ps[:])
                nc.tensor.matmul(y_ps[:], lnT[:], pwt_sb[:, k, :],
                                 start=(k == 0), stop=(k == 1))

            rel_t = ep_pool.tile([128, DIM], F32, tag="rel")
            nc.sync.dma_start(rel_t[:], relsl_d[t * 128:(t + 1) * 128, :])
            delta = ep_pool.tile([128, DIM], F32, tag="delta")
            nc.vector.tensor_scalar(delta[:], y_ps[:], ALPHA, None,
                                    AluOpType.mult)
            out_t = ep_pool.tile([128, DIM], F32, tag="out")
            nc.vector.tensor_tensor(out_t[:], delta[:], rel_t[:],
                                    AluOpType.add)
            nc.sync.dma_start(out_d[t * 128:(t + 1) * 128, :], out_t[:])

    nc.compile()
    return nc


def _prep(rel_embed, rel_edge_index, rel_edge_weight, proj_w):
    """Host-side sharding/layout: scatter edges into dense per-(dst tile,
    src tile) weight blocks; lay out rel_embed for SBUF residency."""
    src = np.asarray(rel_edge_index[0], dtype=np.int64)
    dst = np.asarray(rel_edge_index[1], dtype=np.int64)
    w = np.asarray(rel_edge_weight, dtype=np.float32)
    rel = np.asarray(rel_embed, dtype=np.float32)
    pw = np.asarray(proj_w, dtype=np.float32)

    core = dst // NODES_PER_CORE
    drel = dst - core * NODES_PER_CORE
    t = drel // TILE
    d = drel % TILE
    s = src // TILE
    p = src % TILE
    # flat index inside one core's [N_TILES, S_TILES, 128, 128] block array
    flat = ((t * S_TILES + s) * TILE + p) * TILE + d
    blk_sz = N_TILES * S_TILES * TILE * TILE

    w_dev = np.empty((N_CORES, N_TILES, 128, S_TILES * TILE),
                     dtype=ml_dtypes.bfloat16)
    for c in range(N_CORES):
        m = core == c
        wc = np.bincount(flat[m], weights=w[m], minlength=blk_sz)
        wc = wc.reshape(N_TILES, S_TILES, TILE, TILE).astype(np.float32)
        # -> [t, p(src), s*128+d(dst)] so the SBUF tile is partition=src
        w_dev[c] = wc.transpose(0, 2, 1, 3).reshape(
            N_TILES, 128, S_TILES * TILE)

    rel16 = rel.astype(ml_dtypes.bfloat16)
    rel16_pad = np.zeros((S_TILES * TILE, DIM), dtype=ml_dtypes.bfloat16)
    rel16_pad[:N_NODES] = rel16
    x_dev = np.ascontiguousarray(
        rel16_pad.reshape(S_TILES, TILE, DIM).transpose(1, 0, 2).reshape(
            128, S_TILES * DIM))

    relslice = np.zeros((N_CORES, OUT_ROWS, DIM), dtype=np.float32)
    for c in range(N_CORES):
        relslice[c, :NODES_PER_CORE] = rel[c * NODES_PER_CORE:
                                           (c + 1) * NODES_PER_CORE]
    pwt = pw.T.astype(ml_dtypes.bfloat16)  # [f, o]
    pwt_dev = np.ascontiguousarray(
        pwt.reshape(2, 128, DIM).transpose(1, 0, 2).reshape(128, 2 * DIM))

    in_maps = []
    for c in range(N_CORES):
        in_maps.append({
            "x": x_dev,
            "wblk": w_dev[c],
            "relslice": relslice[c],
            "projwT": pwt_dev,
        })
    return in_maps


def kernel(rel_embed, rel_edge_index, rel_edge_weight, proj_w,
           _trace=False):
    in_maps = _prep(rel_embed, rel_edge_index, rel_edge_weight, proj_w)
    nc = _CACHE.get("nc")
    if nc is None:
        nc = _build()
        _CACHE["nc"] = nc
    res = run_bass_kernel_spmd(nc, in_maps, core_ids=list(range(N_CORES)),
                               trace=_trace)
    out = np.concatenate(
        [res.results[c]["out"][:NODES_PER_CORE] for c in range(N_CORES)],
        axis=0)
    if _trace:
        kernel.last_results = res
    return out.astype(np.float32)


# revision 8
# speedup vs baseline: 3.2188x; 1.0515x over previous
"""DualRelGCN message-passing kernel for 8 TRN2 NeuronCores.

Strategy (destination-sharded, collective-free, block-dense):
  - LayerNorm is invariant to positive per-row scaling, so LN(agg/denom) ==
    LN(agg): the denominator drops out of the computation entirely.
  - Shard edges by dst range: core c owns nodes [1250c, 1250(c+1)) and
    receives every edge whose dst falls there.  Each core computes its 1250
    output rows completely locally -> no collectives.
  - The weighted gather+segment_sum is expressed as a block matmul:
    agg[tile t] = sum_s W_ts.T @ X_s, where W_ts is the [128 src, 128 dst]
    dense block of the weighted adjacency (host-scattered from the edge
    list; ~5% nnz but dense matmul on PE beats any descriptor-generated
    gather path by a wide margin) and X_s is a [128, 256] tile of rel_embed
    (bf16, fully resident in SBUF).  PSUM accumulates over s in fp32.
  - Epilogue per dst tile: LN on DVE/ACT, PE transpose, y = ln @ proj_w.T,
    out = rel_embed + 0.1*y.
  - The device program is fully static: the edge distribution only changes
    tensor *contents*, never the instruction stream.
"""

import sys

for _p in ("/opt/trn_rl_repo",):
    if _p not in sys.path:
        sys.path.insert(0, _p)

from contextlib import ExitStack

import numpy as np
import ml_dtypes

import concourse.bacc as bacc
import concourse.mybir as mybir
from concourse.alu_op_type import AluOpType
from concourse.tile import TileContext
from concourse.bass_utils import run_bass_kernel_spmd

F32 = mybir.dt.float32
BF16 = mybir.dt.bfloat16
FP8 = mybir.dt.float8e4
AF = mybir.ActivationFunctionType

N_NODES = 10000
DIM = 256
N_CORES = 8
NODES_PER_CORE = N_NODES // N_CORES  # 1250
TILE = 128
N_TILES = -(-NODES_PER_CORE // TILE)  # 10 dst tiles per core
S_TILES = -(-N_NODES // TILE)  # 79 src tiles
OUT_ROWS = N_TILES * TILE  # 1280
ALPHA = 0.1
LN_EPS = 1e-5
X_CHUNKS = 8  # X load split for pipelined startup
W_CHUNKS = 4  # W load split per dst tile

_CACHE: dict = {}


def _build():
    nc = bacc.Bacc("TRN2", target_bir_lowering=False, debug=False,
                   num_devices=N_CORES)

    x_d = nc.dram_tensor("x", [128, S_TILES * DIM], BF16, kind="ExternalInput")
    w_d = nc.dram_tensor("wblk", [N_TILES, 128, S_TILES * TILE], FP8,
                         kind="ExternalInput")
    relsl_d = nc.dram_tensor("relslice", [OUT_ROWS, DIM], F32,
                             kind="ExternalInput")
    pwt_d = nc.dram_tensor("projwT", [128, 2 * DIM], BF16,
                           kind="ExternalInput")
    out_d = nc.dram_tensor("out", [OUT_ROWS, DIM], F32, kind="ExternalOutput")

    with TileContext(nc) as tc, ExitStack() as es:
        const_pool = es.enter_context(tc.tile_pool(name="const", bufs=1))
        wpool = es.enter_context(tc.tile_pool(name="wblk", bufs=2))
        ep_pool = es.enter_context(tc.tile_pool(name="ep", bufs=2))
        ps_agg = es.enter_context(tc.tile_pool(name="ps_agg", bufs=2,
                                               space="PSUM"))
        ps_tr = es.enter_context(tc.tile_pool(name="ps_tr", bufs=2,
                                              space="PSUM"))
        ps_y = es.enter_context(tc.tile_pool(name="ps_y", bufs=2,
                                             space="PSUM"))

        # --- constants / resident inputs ---
        iota_row = const_pool.tile([128, 128], F32, tag="iota")
        nc.gpsimd.iota(iota_row[:], [[1, 128]], base=0, channel_multiplier=0,
                       allow_small_or_imprecise_dtypes=True)
        pidx = const_pool.tile([128, 1], F32, tag="pidx")
        nc.gpsimd.iota(pidx[:], [[1, 1]], base=0, channel_multiplier=1,
                       allow_small_or_imprecise_dtypes=True)
        ident = const_pool.tile([128, 128], BF16, tag="ident")
        nc.vector.tensor_scalar(ident[:], iota_row[:], pidx[:], None,
                                AluOpType.is_equal)
        epsb = const_pool.tile([128, 1], F32, tag="epsb")
        nc.vector.memset(epsb[:], LN_EPS)
        pwt_sb = const_pool.tile([128, 2, DIM], BF16, tag="pwt")
        nc.sync.dma_start(pwt_sb[:], pwt_d[:])

        # rel_embed (bf16), fully resident; chunked load so dst-tile 0's
        # matmuls can start before the whole 5 MB lands
        x_sb = const_pool.tile([128, S_TILES, DIM], BF16, tag="x")
        bounds = [round(i * S_TILES / X_CHUNKS) for i in range(X_CHUNKS + 1)]
        for i in range(X_CHUNKS):
            lo, hi = bounds[i], bounds[i + 1]
            nc.sync.dma_start(x_sb[:, lo:hi, :],
                              x_d[:, lo * DIM:hi * DIM])

        wb = [round(i * S_TILES / W_CHUNKS) for i in range(W_CHUNKS + 1)]
        for t in range(N_TILES):
            w_t = wpool.tile([128, S_TILES, TILE], FP8, tag="w")
            for i in range(W_CHUNKS):
                lo, hi = wb[i], wb[i + 1]
                nc.sync.dma_start(w_t[:, lo:hi, :],
                                  w_d[t, :, lo * TILE:hi * TILE])

            agg_ps = ps_agg.tile([128, DIM], F32, tag="agg")
            for s in range(S_TILES):
                nc.tensor.matmul(agg_ps[:], w_t[:, s, :], x_sb[:, s, :],
                                 start=(s == 0), stop=(s == S_TILES - 1))

            # --- epilogue: LN -> transpose -> @ proj_w.T -> residual ---
            agg = ep_pool.tile([128, DIM], F32, tag="agg_sb")
            rowsum = ep_pool.tile([128, 1], F32, tag="rowsum")
            nc.scalar.activation(agg[:], agg_ps[:], AF.Copy,
                                 accum_out=rowsum[:])
            mean = ep_pool.tile([128, 1], F32, tag="mean")
            nc.scalar.mul(mean[:], rowsum[:], 1.0 / DIM)
            cent = ep_pool.tile([128, DIM], F32, tag="cent")
            nc.vector.tensor_scalar(cent[:], agg[:], mean[:], None,
                                    AluOpType.subtract)
            sq = ep_pool.tile([128, DIM], F32, tag="sq")
            sumsq = ep_pool.tile([128, 1], F32, tag="sumsq")
            nc.scalar.activation(sq[:], cent[:], AF.Square,
                                 accum_out=sumsq[:])
            std = ep_pool.tile([128, 1], F32, tag="std")
            nc.scalar.activation(std[:], sumsq[:], AF.Sqrt, bias=epsb[:],
                                 scale=1.0 / DIM)
            rstd = ep_pool.tile([128, 1], F32, tag="rstd")
            nc.vector.reciprocal(rstd[:], std[:])
            ln = ep_pool.tile([128, DIM], BF16, tag="ln")
            nc.vector.tensor_scalar(ln[:], cent[:], rstd[:], None,
                                    AluOpType.mult)

            y_ps = ps_y.tile([128, DIM], F32, tag="y")
            for k in range(2):
                tr_ps = ps_tr.tile([128, 128], BF16, tag="tr")
                nc.tensor.transpose(tr_ps[:], ln[:, k * 128:(k + 1) * 128],
                                    ident[:])
                lnT = ep_pool.tile([128, 128], BF16, tag="lnT")
                nc.scalar.copy(lnT[:], tr_ps[:])
                nc.tensor.matmul(y_ps[:], lnT[:], pwt_sb[:, k, :],
                                 start=(k == 0), stop=(k == 1))

            rel_t = ep_pool.tile([128, DIM], F32, tag="rel")
            nc.sync.dma_start(rel_t[:], relsl_d[t * 128:(t + 1) * 128, :])
            delta = ep_pool.tile([128, DIM], F32, tag="delta")
            nc.vector.tensor_scalar(delta[:], y_ps[:], ALPHA, None,
                                    AluOpType.mult)
            out_t = ep_pool.tile([128, DIM], F32, tag="out")
            nc.vector.tensor_tensor(out_t[:], delta[:], rel_t[:],
                                    AluOpType.add)
            nc.sync.dma_start(out_d[t * 128:(t + 1) * 128, :], out_t[:])

    nc.compile()
    return nc


def _prep(rel_embed, rel_edge_index, rel_edge_weight, proj_w):
    """Host-side sharding/layout: scatter edges into dense per-(dst tile,
    src tile) weight blocks; lay out rel_embed for SBUF residency."""
    src = np.asarray(rel_edge_index[0], dtype=np.int64)
    dst = np.asarray(rel_edge_index[1], dtype=np.int64)
    w = np.asarray(rel_edge_weight, dtype=np.float32)
    rel = np.asarray(rel_embed, dtype=np.float32)
    pw = np.asarray(proj_w, dtype=np.float32)

    core = dst // NODES_PER_CORE
    drel = dst - core * NODES_PER_CORE
    t = drel // TILE
    d = drel % TILE
    s = src // TILE
    p = src % TILE
    # flat index inside one core's [N_TILES, S_TILES, 128, 128] block array
    flat = ((t * S_TILES + s) * TILE + p) * TILE + d
    blk_sz = N_TILES * S_TILES * TILE * TILE

    w_dev = np.empty((N_CORES, N_TILES, 128, S_TILES * TILE),
                     dtype=ml_dtypes.float8_e4m3)
    for c in range(N_CORES):
        m = core == c
        wc = np.bincount(flat[m], weights=w[m], minlength=blk_sz)
        wc = wc.reshape(N_TILES, S_TILES, TILE, TILE).astype(np.float32)
        # -> [t, p(src), s*128+d(dst)] so the SBUF tile is partition=src
        w_dev[c] = wc.transpose(0, 2, 1, 3).reshape(
            N_TILES, 128, S_TILES * TILE)

    rel16 = rel.astype(ml_dtypes.bfloat16)
    rel16_pad = np.zeros((S_TILES * TILE, DIM), dtype=ml_dtypes.bfloat16)
    rel16_pad[:N_NODES] = rel16
    x_dev = np.ascontiguousarray(
        rel16_pad.reshape(S_TILES, TILE, DIM).transpose(1, 0, 2).reshape(
            128, S_TILES * DIM))

    relslice = np.zeros((N_CORES, OUT_ROWS, DIM), dtype=np.float32)
    for c in range(N_CORES):
        relslice[c, :NODES_PER_CORE] = rel[c * NODES_PER_CORE:
                                           (c + 1) * NODES_PER_CORE]
    pwt = pw.T.astype(ml_dtypes.bfloat16)  # [f, o]
    pwt_dev = np.ascontiguousarray(
        pwt.reshape(2, 128, DIM).transpose(1, 0, 2).reshape(128, 2 * DIM))

    in_maps = []
    for c in range(N_CORES):
        in_maps.append({
            "x": x_dev,
            "wblk": w_dev[c],
            "relslice": relslice[c],
            "projwT": pwt_dev,
        })
    return in_maps


def kernel(rel_embed, rel_edge_index, rel_edge_weight, proj_w,
           _trace=False):
    in_maps = _prep(rel_embed, rel_edge_index, rel_edge_weight, proj_w)
    nc = _CACHE.get("nc")
    if nc is None:
        nc = _build()
        _CACHE["nc"] = nc
    res = run_bass_kernel_spmd(nc, in_maps, core_ids=list(range(N_CORES)),
                               trace=_trace)
    out = np.concatenate(
        [res.results[c]["out"][:NODES_PER_CORE] for c in range(N_CORES)],
        axis=0)
    if _trace:
        kernel.last_results = res
    return out.astype(np.float32)


# revision 10
# speedup vs baseline: 3.3665x; 1.0459x over previous
"""DualRelGCN message-passing kernel for 8 TRN2 NeuronCores.

Strategy (destination-sharded, collective-free, block-dense):
  - LayerNorm is invariant to positive per-row scaling, so LN(agg/denom) ==
    LN(agg): the denominator drops out of the computation entirely.
  - Shard edges by dst range: core c owns nodes [1250c, 1250(c+1)) and
    receives every edge whose dst falls there.  Each core computes its 1250
    output rows completely locally -> no collectives.
  - The weighted gather+segment_sum is expressed as a block matmul:
    agg[tile t] = sum_s W_ts.T @ X_s, where W_ts is the [128 src, 128 dst]
    dense block of the weighted adjacency (host-scattered from the edge
    list; ~5% nnz but dense matmul on PE beats any descriptor-generated
    gather path by a wide margin) and X_s is a [128, 256] tile of rel_embed
    (bf16, fully resident in SBUF).  PSUM accumulates over s in fp32.
  - Epilogue per dst tile: LN on DVE/ACT, PE transpose, y = ln @ proj_w.T,
    out = rel_embed + 0.1*y.
  - The device program is fully static: the edge distribution only changes
    tensor *contents*, never the instruction stream.
"""

import sys

for _p in ("/opt/trn_rl_repo",):
    if _p not in sys.path:
        sys.path.insert(0, _p)

from contextlib import ExitStack

import numpy as np
import ml_dtypes

import concourse.bacc as bacc
import concourse.mybir as mybir
from concourse.alu_op_type import AluOpType
from concourse.tile import TileContext
from concourse.bass_utils import run_bass_kernel_spmd

F32 = mybir.dt.float32
BF16 = mybir.dt.bfloat16
FP8 = mybir.dt.float8e4
AF = mybir.ActivationFunctionType

N_NODES = 10000
DIM = 256
N_CORES = 8
NODES_PER_CORE = N_NODES // N_CORES  # 1250
TILE = 128
N_TILES = -(-NODES_PER_CORE // TILE)  # 10 dst tiles per core
S_TILES = -(-N_NODES // TILE)  # 79 src tiles
OUT_ROWS = N_TILES * TILE  # 1280
ALPHA = 0.1
LN_EPS = 1e-5
X_CHUNKS = 8  # X load split for pipelined startup
W_CHUNKS = 4  # W load split per dst tile

_CACHE: dict = {}


def _build():
    nc = bacc.Bacc("TRN2", target_bir_lowering=False, debug=False,
                   num_devices=N_CORES)

    x_d = nc.dram_tensor("x", [128, S_TILES * DIM], BF16, kind="ExternalInput")
    w_d = nc.dram_tensor("wblk", [N_TILES, 128, S_TILES * TILE], FP8,
                         kind="ExternalInput")
    relsl_d = nc.dram_tensor("relslice", [OUT_ROWS, DIM], F32,
                             kind="ExternalInput")
    pwt_d = nc.dram_tensor("projwT", [128, 2 * DIM], BF16,
                           kind="ExternalInput")
    out_d = nc.dram_tensor("out", [OUT_ROWS, DIM], F32, kind="ExternalOutput")

    with TileContext(nc) as tc, ExitStack() as es:
        const_pool = es.enter_context(tc.tile_pool(name="const", bufs=1))
        wpool = es.enter_context(tc.tile_pool(name="wblk", bufs=2))
        ep_pool = es.enter_context(tc.tile_pool(name="ep", bufs=2))
        ps_agg = es.enter_context(tc.tile_pool(name="ps_agg", bufs=2,
                                               space="PSUM"))
        ps_tr = es.enter_context(tc.tile_pool(name="ps_tr", bufs=2,
                                              space="PSUM"))
        ps_y = es.enter_context(tc.tile_pool(name="ps_y", bufs=2,
                                             space="PSUM"))

        # --- constants / resident inputs ---
        iota_row = const_pool.tile([128, 128], F32, tag="iota")
        nc.gpsimd.iota(iota_row[:], [[1, 128]], base=0, channel_multiplier=0,
                       allow_small_or_imprecise_dtypes=True)
        pidx = const_pool.tile([128, 1], F32, tag="pidx")
        nc.gpsimd.iota(pidx[:], [[1, 1]], base=0, channel_multiplier=1,
                       allow_small_or_imprecise_dtypes=True)
        ident = const_pool.tile([128, 128], BF16, tag="ident")
        nc.vector.tensor_scalar(ident[:], iota_row[:], pidx[:], None,
                                AluOpType.is_equal)
        epsb = const_pool.tile([128, 1], F32, tag="epsb")
        nc.vector.memset(epsb[:], LN_EPS)
        pwt_sb = const_pool.tile([128, 2, DIM], BF16, tag="pwt")
        nc.sync.dma_start(pwt_sb[:], pwt_d[:])

        # rel_embed (bf16), fully resident; chunked load so dst-tile 0's
        # matmuls can start before the whole 5 MB lands.  X rides the
        # scalar-engine HWDGE ring so it doesn't queue ahead of W's
        # sync-engine ring (per-engine FIFO).
        x_sb = const_pool.tile([128, S_TILES, DIM], BF16, tag="x")
        bounds = [round(i * S_TILES / X_CHUNKS) for i in range(X_CHUNKS + 1)]
        for i in range(X_CHUNKS):
            lo, hi = bounds[i], bounds[i + 1]
            nc.scalar.dma_start(x_sb[:, lo:hi, :],
                                x_d[:, lo * DIM:hi * DIM])

        wb = [round(i * S_TILES / W_CHUNKS) for i in range(W_CHUNKS + 1)]
        for t in range(N_TILES):
            w_t = wpool.tile([128, S_TILES, TILE], FP8, tag="w")
            for i in range(W_CHUNKS):
                lo, hi = wb[i], wb[i + 1]
                nc.sync.dma_start(w_t[:, lo:hi, :],
                                  w_d[t, :, lo * TILE:hi * TILE])

            agg_ps = ps_agg.tile([128, DIM], F32, tag="agg")
            for s in range(S_TILES):
                nc.tensor.matmul(agg_ps[:], w_t[:, s, :], x_sb[:, s, :],
                                 start=(s == 0), stop=(s == S_TILES - 1))

            # --- epilogue: LN -> transpose -> @ proj_w.T -> residual ---
            agg = ep_pool.tile([128, DIM], F32, tag="agg_sb")
            rowsum = ep_pool.tile([128, 1], F32, tag="rowsum")
            nc.scalar.activation(agg[:], agg_ps[:], AF.Copy,
                                 accum_out=rowsum[:])
            mean = ep_pool.tile([128, 1], F32, tag="mean")
            nc.scalar.mul(mean[:], rowsum[:], 1.0 / DIM)
            cent = ep_pool.tile([128, DIM], F32, tag="cent")
            nc.vector.tensor_scalar(cent[:], agg[:], mean[:], None,
                                    AluOpType.subtract)
            sq = ep_pool.tile([128, DIM], F32, tag="sq")
            sumsq = ep_pool.tile([128, 1], F32, tag="sumsq")
            nc.scalar.activation(sq[:], cent[:], AF.Square,
                                 accum_out=sumsq[:])
            std = ep_pool.tile([128, 1], F32, tag="std")
            nc.scalar.activation(std[:], sumsq[:], AF.Sqrt, bias=epsb[:],
                                 scale=1.0 / DIM)
            rstd = ep_pool.tile([128, 1], F32, tag="rstd")
            nc.vector.reciprocal(rstd[:], std[:])
            ln = ep_pool.tile([128, DIM], BF16, tag="ln")
            nc.vector.tensor_scalar(ln[:], cent[:], rstd[:], None,
                                    AluOpType.mult)

            y_ps = ps_y.tile([128, DIM], F32, tag="y")
            for k in range(2):
                tr_ps = ps_tr.tile([128, 128], BF16, tag="tr")
                nc.tensor.transpose(tr_ps[:], ln[:, k * 128:(k + 1) * 128],
                                    ident[:])
                lnT = ep_pool.tile([128, 128], BF16, tag="lnT")
                nc.scalar.copy(lnT[:], tr_ps[:])
                nc.tensor.matmul(y_ps[:], lnT[:], pwt_sb[:, k, :],
                                 start=(k == 0), stop=(k == 1))

            rel_t = ep_pool.tile([128, DIM], F32, tag="rel")
            nc.scalar.dma_start(rel_t[:], relsl_d[t * 128:(t + 1) * 128, :])
            delta = ep_pool.tile([128, DIM], F32, tag="delta")
            nc.vector.tensor_scalar(delta[:], y_ps[:], ALPHA, None,
                                    AluOpType.mult)
            out_t = ep_pool.tile([128, DIM], F32, tag="out")
            nc.vector.tensor_tensor(out_t[:], delta[:], rel_t[:],
                                    AluOpType.add)
            nc.sync.dma_start(out_d[t * 128:(t + 1) * 128, :], out_t[:])

    nc.compile()
    return nc


def _prep(rel_embed, rel_edge_index, rel_edge_weight, proj_w):
    """Host-side sharding/layout: scatter edges into dense per-(dst tile,
    src tile) weight blocks; lay out rel_embed for SBUF residency."""
    src = np.asarray(rel_edge_index[0], dtype=np.int64)
    dst = np.asarray(rel_edge_index[1], dtype=np.int64)
    w = np.asarray(rel_edge_weight, dtype=np.float32)
    rel = np.asarray(rel_embed, dtype=np.float32)
    pw = np.asarray(proj_w, dtype=np.float32)

    core = dst // NODES_PER_CORE
    drel = dst - core * NODES_PER_CORE
    t = drel // TILE
    d = drel % TILE
    s = src // TILE
    p = src % TILE
    # flat index inside one core's [N_TILES, S_TILES, 128, 128] block array
    flat = ((t * S_TILES + s) * TILE + p) * TILE + d
    blk_sz = N_TILES * S_TILES * TILE * TILE

    w_dev = np.empty((N_CORES, N_TILES, 128, S_TILES * TILE),
                     dtype=ml_dtypes.float8_e4m3)
    for c in range(N_CORES):
        m = core == c
        wc = np.bincount(flat[m], weights=w[m], minlength=blk_sz)
        wc = wc.reshape(N_TILES, S_TILES, TILE, TILE).astype(np.float32)
        # -> [t, p(src), s*128+d(dst)] so the SBUF tile is partition=src
        w_dev[c] = wc.transpose(0, 2, 1, 3).reshape(
            N_TILES, 128, S_TILES * TILE)

    rel16 = rel.astype(ml_dtypes.bfloat16)
    rel16_pad = np.zeros((S_TILES * TILE, DIM), dtype=ml_dtypes.bfloat16)
    rel16_pad[:N_NODES] = rel16
    x_dev = np.ascontiguousarray(
        rel16_pad.reshape(S_TILES, TILE, DIM).transpose(1, 0, 2).reshape(
            128, S_TILES * DIM))

    relslice = np.zeros((N_CORES, OUT_ROWS, DIM), dtype=np.float32)
    for c in range(N_CORES):
        relslice[c, :NODES_PER_CORE] = rel[c * NODES_PER_CORE:
                                           (c + 1) * NODES_PER_CORE]
    pwt = pw.T.astype(ml_dtypes.bfloat16)  # [f, o]
    pwt_dev = np.ascontiguousarray(
        pwt.reshape(2, 128, DIM).transpose(1, 0, 2).reshape(128, 2 * DIM))

    in_maps = []
    for c in range(N_CORES):
        in_maps.append({
            "x": x_dev,
            "wblk": w_dev[c],
            "relslice": relslice[c],
            "projwT": pwt_dev,
        })
    return in_maps


def kernel(rel_embed, rel_edge_index, rel_edge_weight, proj_w,
           _trace=False):
    in_maps = _prep(rel_embed, rel_edge_index, rel_edge_weight, proj_w)
    nc = _CACHE.get("nc")
    if nc is None:
        nc = _build()
        _CACHE["nc"] = nc
    res = run_bass_kernel_spmd(nc, in_maps, core_ids=list(range(N_CORES)),
                               trace=_trace)
    out = np.concatenate(
        [res.results[c]["out"][:NODES_PER_CORE] for c in range(N_CORES)],
        axis=0)
    if _trace:
        kernel.last_results = res
    return out.astype(np.float32)
